# revision 17
# baseline (speedup 1.0000x reference)
"""GQA forward (b=2, s=2048, H=32 q heads, 8 kv heads, d=64) on 8 TRN2 cores.

Sharding: core k owns query heads 4k..4k+3 and kv head k. GQA group
structure makes attention fully local per core (q heads 4k..4k+3 attend
only to kv head k). x is replicated; W columns are sharded; outputs are
column-concatenated.

Per-core kernel (Tile framework), fp16 datapath / fp32 accumulation:
  - x.T is produced on the HOST (numpy transpose + fp16 cast) and DMA'd
    straight into SBUF — no on-chip transposes of x.
  - W-stationary projections: QKV.T[wcol, s] = W_chunk.T @ xT accumulated
    in fp32 PSUM over 16 k-chunks, 512-wide moving — Q.T/K.T come out
    already in [d, s] layout, no flip transposes. W columns are permuted
    on the host so each head's rows are [evens(32) | odds(32)] (RoPE
    pairs de-interleaved; scores are invariant to a shared d-permutation
    of Q and K).
  - RoPE on DVE in [d, s] layout: partner rows fetched with 32-partition
    cross-quadrant copies, then out = ppt*cos + partner*sin(signed) as
    three [128, 512] tensor_tensor ops fused with the PSUM->SBUF
    eviction (V rows pass through via cos=1/sin=0 table entries).
  - V.T flipped back to [kv, d] via 4 small PE transposes per s-tile.
  - Attention in transposed layout: S.T[kv,q] = K @ Q.T per 128-kv block,
    exp on ACT (scale=1/8 folded in) with fp16 output, causal handled by
    skipping blocks above the diagonal + multiplying the diagonal block
    of P by a 0/1 fp16 mask, ctx.T[65,q] = [V|1].T @ P.T accumulated in
    fp32 PSUM (row 64 = softmax sums).
  - Finalize: 4 PE transposes of ctx.T into one [128, 4x66] PSUM tile
    (shared with the cxt pool), one reciprocal, 4 scalar muls, one DMA
    per (head, s-tile).
"""

import numpy as np
from contextlib import ExitStack

import concourse.bass as bass
import concourse.bacc as bacc
import concourse.mybir as mybir
from concourse import tile
from concourse.bass_utils import run_bass_kernel_spmd

F32 = mybir.dt.float32
F16 = mybir.dt.float16
MUL = mybir.AluOpType.mult
ADD = mybir.AluOpType.add

B = 2
S = 2048
DIN = 2048
D = 64              # head dim
HPC = 4             # query heads per core
NCORES = 8
WCOLS = 4 * D + D + D  # 256 q cols + 64 k + 64 v = 384
ST = 512            # s-tile (rows per outer step)
NST = B * S // ST   # 8 s-tiles
NCH = DIN // 128    # 16 k-chunks
NKV = S // 128      # kv tiles per batch


def build_bass():
    nc = bacc.Bacc(None, target_bir_lowering=False)
    xt_d = nc.declare_dram_parameter("xt", [DIN, B * S], F16, isOutput=False)
    w_d = nc.declare_dram_parameter("w", [DIN, WCOLS], F16, isOutput=False)
    cq_d = nc.declare_dram_parameter("cq", [128, S], F16, isOutput=False)
    sq_d = nc.declare_dram_parameter("sq", [128, S], F16, isOutput=False)
    ck_d = nc.declare_dram_parameter("ck", [128, S], F16, isOutput=False)
    sk_d = nc.declare_dram_parameter("sk", [128, S], F16, isOutput=False)
    mask_d = nc.declare_dram_parameter("mask", [128, 128], F16, isOutput=False)
    out_d = nc.declare_dram_parameter("out", [B * S, HPC * D], F32, isOutput=True)

    with ExitStack() as ctx:
        tc = ctx.enter_context(tile.TileContext(nc))
        const = ctx.enter_context(tc.tile_pool(name="const", bufs=1))
        resid = ctx.enter_context(tc.tile_pool(name="resid", bufs=1))
        xt_p = ctx.enter_context(tc.tile_pool(name="xt", bufs=2))
        ro_p = ctx.enter_context(tc.tile_pool(name="ro", bufs=2))
        qt_p = ctx.enter_context(tc.tile_pool(name="qt", bufs=4))
        kvq_p = ctx.enter_context(tc.tile_pool(name="kvq", bufs=2))
        p_p = ctx.enter_context(tc.tile_pool(name="p", bufs=3))
        cx_p = ctx.enter_context(tc.tile_pool(name="cx", bufs=2))
        o_p = ctx.enter_context(tc.tile_pool(name="o", bufs=3))
        rv_p = ctx.enter_context(tc.tile_pool(name="rv", bufs=3))
        pr_ps = ctx.enter_context(tc.tile_pool(name="pr_ps", bufs=3, space="PSUM"))
        sc_ps = ctx.enter_context(tc.tile_pool(name="sc_ps", bufs=3, space="PSUM"))
        cx_ps = ctx.enter_context(tc.tile_pool(name="cx_ps", bufs=2, space="PSUM"))

        # constants on the SCALAR engine's DMA queue so the x.T tiles (sync
        # queue) stream in parallel — the first projection only waits for
        # w chunk-group 0 + xt chunk-group 0 instead of ~7MB of constants.
        w_sb = const.tile([128, NCH, WCOLS], F16)
        for cg in range(4):
            nc.scalar.dma_start(
                out=w_sb[:, cg * 4:(cg + 1) * 4, :],
                in_=w_d[cg * 512:(cg + 1) * 512, :].rearrange(
                    "(c p) n -> p c n", p=128))
        mask_sb = const.tile([128, 128], F16)
        nc.scalar.dma_start(out=mask_sb[:], in_=mask_d[:])
        # rope tables in [d-row, s] layout, fp16; ~4KB/partition each
        cq = const.tile([128, S], F16)
        nc.scalar.dma_start(out=cq[:], in_=cq_d[:])
        sq = const.tile([128, S], F16)
        nc.scalar.dma_start(out=sq[:], in_=sq_d[:])
        ck = const.tile([128, S], F16)
        nc.scalar.dma_start(out=ck[:], in_=ck_d[:])
        sk = const.tile([128, S], F16)
        nc.scalar.dma_start(out=sk[:], in_=sk_d[:])

        # rows 0-63: K.T (RoPE'd); rows 64-127: duplicate copy so that the
        # scores matmul lhsT can match either base partition of the Q halves
        kt_res = resid.tile([128, B * S], F16)
        vp_res = resid.tile([128, B * NKV, 128], F16)  # [V|1|0pad] kv-tiles
        nc.vector.memset(vp_res[:], 0.0)
        nc.vector.memset(vp_res[:, :, 64:65], 1.0)

        for st in range(NST):
            b, sti = divmod(st, 4)
            ssl = slice(sti * ST, (sti + 1) * ST)  # within-batch s range

            # ---- x.T tile straight from HBM (host-transposed), split so the
            # first chunk-group's projections can start before the rest land
            xt = xt_p.tile([128, NCH, ST], F16, tag="xt")
            for cg in range(4):
                nc.sync.dma_start(
                    out=xt[:, cg * 4:(cg + 1) * 4, :],
                    in_=xt_d[cg * 512:(cg + 1) * 512,
                             st * ST:(st + 1) * ST].rearrange(
                                 "(c p) s -> p c s", p=128))

            # ---- W-stationary projections + RoPE in [d, s] layout ----
            # wb 0: [h0_e h0_o h1_e h1_o], wb 1: [h2_e h2_o h3_e h3_o],
            # wb 2: [k_e k_o | V(64 natural cols, transposed layout)]
            qta = qt_p.tile([128, ST], F16, tag="qta")
            qtb = qt_p.tile([128, ST], F16, tag="qtb")
            kvq = kvq_p.tile([128, ST], F16, tag="kvq")
            for wb in range(3):
                ppt = pr_ps.tile([128, ST], F32, tag="ppt")
                for c in range(NCH):
                    nc.tensor.matmul(
                        ppt[:], w_sb[:, c, wb * 128:(wb + 1) * 128],
                        xt[:, c, :], start=(c == 0), stop=(c == NCH - 1))
                # partner rows for the rotation (32-part cross-quadrant
                # copies), then qdst = ppt*C + partner*S_signed
                sh = ro_p.tile([128, ST], F32, tag="sh")
                nc.vector.tensor_copy(sh[0:32, :], ppt[32:64, :])
                nc.vector.tensor_copy(sh[32:64, :], ppt[0:32, :])
                if wb < 2:
                    nc.vector.tensor_copy(sh[64:96, :], ppt[96:128, :])
                    nc.vector.tensor_copy(sh[96:128, :], ppt[64:96, :])
                    qdst, tc_, ts_ = (qta if wb == 0 else qtb), cq, sq
                else:
                    # V rows pass through (ck=1, sk=0); keep sh initialized
                    nc.vector.tensor_copy(sh[64:128, :], ppt[64:128, :])
                    qdst, tc_, ts_ = kvq, ck, sk
                ts2 = ro_p.tile([128, ST], F32, tag="ts2")
                nc.vector.tensor_tensor(qdst[:], ppt[:], tc_[:, ssl], MUL)
                nc.vector.tensor_tensor(ts2[:], sh[:], ts_[:, ssl], MUL)
                nc.vector.tensor_tensor(qdst[:], qdst[:], ts2[:], ADD)
            # K.T rows into the resident (plus base-64 duplicate via DMA)
            nc.vector.tensor_copy(
                kt_res[0:64, st * ST:(st + 1) * ST], kvq[0:64, :])
            nc.sync.dma_start(
                out=kt_res[64:128, st * ST:(st + 1) * ST],
                in_=kt_res[0:64, st * ST:(st + 1) * ST])
            # V.T -> [kv, d] natural via XBAR DMA transpose, straight into
            # the resident (off the PE)
            for vc in range(4):
                nc.sync.dma_start_transpose(
                    out=vp_res[:, b * NKV + sti * 4 + vc, 0:64],
                    in_=kvq[64:128, vc * 128:(vc + 1) * 128])

            # ---- attention for the 4 heads of this q-tile ----
            js = [4 * sti] + list(range(4 * sti)) + \
                 [4 * sti + 1, 4 * sti + 2, 4 * sti + 3]
            for h in range(HPC):
                p0 = (h % 2) * 64
                qh = (qta if h < 2 else qtb)[p0:p0 + 64, :]
                cxt = cx_ps.tile([128, ST], F32, tag="cxt")
                for idx, j in enumerate(js):
                    off = 128 * j - 512 * sti
                    w0 = max(0, off)
                    sc = sc_ps.tile([128, ST], F32, tag="sc")
                    nc.tensor.matmul(
                        sc[:, w0:ST],
                        kt_res[p0:p0 + 64, b * S + j * 128:b * S + (j + 1) * 128],
                        qh[:, w0:ST], start=True, stop=True)
                    psb = p_p.tile([128, ST], F16, tag="psb")
                    nc.scalar.activation(
                        psb[:, w0:ST], sc[:, w0:ST],
                        mybir.ActivationFunctionType.Exp, scale=0.125)
                    if j >= 4 * sti:
                        # zero the upper-triangle of the diagonal block
                        nc.vector.tensor_tensor(
                            psb[:, off:off + 128], psb[:, off:off + 128],
                            mask_sb[:], MUL)
                    nc.tensor.matmul(
                        cxt[:, w0:ST], vp_res[:, b * NKV + j, :],
                        psb[:, w0:ST],
                        start=(idx == 0), stop=(idx == len(js) - 1))
                cxs = cx_p.tile([80, ST], F16, tag="cxs")
                nc.vector.tensor_copy(cxs[:], cxt[0:80, :])
                # transpose ctx.T back to [q, d+sums] via XBAR (off the PE);
                # rows 65-79 are zero padding to meet the 16-partition rule
                fo = cx_p.tile([128, 4, 80], F16, tag="fo")
                for qq in range(4):
                    nc.sync.dma_start_transpose(
                        out=fo[:, qq, :], in_=cxs[:, qq * 128:(qq + 1) * 128])
                rv = rv_p.tile([128, 4], F32, tag="rv")
                nc.vector.reciprocal(rv[:], fo[:, :, 64])
                ob = o_p.tile([128, 4, 64], F32, tag="ob")
                for qq in range(4):
                    nc.vector.tensor_scalar_mul(
                        ob[:, qq, :], fo[:, qq, 0:64], rv[:, qq:qq + 1])
                nc.sync.dma_start(
                    out=out_d[st * ST:(st + 1) * ST,
                              h * 64:(h + 1) * 64].rearrange(
                                  "(q p) d -> p q d", p=128),
                    in_=ob[:])
    return nc


_NC_CACHE = None


def _host_consts():
    i = np.arange(0, D, 2, dtype=np.float64) / D          # 32 pair exponents
    freqs = 1.0 / (10000.0 ** i)                           # (32,)
    ang = np.arange(S, dtype=np.float64)[:, None] * freqs[None, :]  # (S, 32)
    cos32 = np.cos(ang).astype(np.float32).T               # (32, S)
    sin32 = np.sin(ang).astype(np.float32).T
    ones = np.ones((64, S), np.float32)
    zeros = np.zeros((64, S), np.float32)
    cq = np.vstack([cos32, cos32, cos32, cos32]).astype(np.float16)
    sq = np.vstack([-sin32, sin32, -sin32, sin32]).astype(np.float16)
    ck = np.vstack([cos32, cos32, ones]).astype(np.float16)
    sk = np.vstack([-sin32, sin32, zeros]).astype(np.float16)
    kv, qq = np.meshgrid(np.arange(128), np.arange(128), indexing="ij")
    mask01 = (kv <= qq).astype(np.float16)                 # 1 = allowed
    return cq, sq, ck, sk, mask01


def _deint(w):
    # de-interleave rope pairs per 64-col head: [evens | odds]
    return np.hstack([w[:, 0::2], w[:, 1::2]])


def _in_maps(x, Wq, Wk, Wv):
    x = np.asarray(x, dtype=np.float32).reshape(B * S, DIN)
    xt = np.ascontiguousarray(x.T).astype(np.float16)      # [DIN, B*S]
    Wq = np.asarray(Wq, dtype=np.float32)
    Wk = np.asarray(Wk, dtype=np.float32)
    Wv = np.asarray(Wv, dtype=np.float32)
    cq, sq, ck, sk, mask01 = _host_consts()

    in_maps = []
    for k in range(NCORES):
        cols = []
        for h in range(4):
            cols.append(_deint(Wq[:, (4 * k + h) * 64:(4 * k + h + 1) * 64]))
        cols.append(_deint(Wk[:, k * 64:(k + 1) * 64]))
        cols.append(Wv[:, k * 64:(k + 1) * 64])
        w_all = np.hstack(cols).astype(np.float16)
        in_maps.append({
            "xt": xt, "w": np.ascontiguousarray(w_all),
            "cq": cq, "sq": sq, "ck": ck, "sk": sk, "mask": mask01,
        })
    return in_maps


def _run(in_maps, **kwargs):
    global _NC_CACHE
    if _NC_CACHE is None:
        _NC_CACHE = build_bass()
        _NC_CACHE.finalize()
    return run_bass_kernel_spmd(_NC_CACHE, in_maps, list(range(NCORES)),
                                **kwargs)


def kernel(x, Wq, Wk, Wv):
    res = _run(_in_maps(x, Wq, Wk, Wv))
    out = np.concatenate([res.results[k]["out"] for k in range(NCORES)], axis=1)
    return out.reshape(B, S, 32 * D)


# revision 18
# speedup vs baseline: 1.3620x; 1.3620x over previous
"""GQA forward (b=2, s=2048, H=32 q heads, 8 kv heads, d=64) on 8 TRN2 cores.

Sharding: core k owns query heads 4k..4k+3 and kv head k. GQA group
structure makes attention fully local per core (q heads 4k..4k+3 attend
only to kv head k). x is replicated; W columns are sharded; outputs are
column-concatenated.

Per-core kernel (Tile framework), fp16 datapath / fp32 accumulation:
  - x.T is produced on the HOST (numpy transpose + fp16 cast) and DMA'd
    straight into SBUF — no on-chip transposes of x.
  - W-stationary projections: QKV.T[wcol, s] = W_chunk.T @ xT accumulated
    in fp32 PSUM over 16 k-chunks, 512-wide moving — Q.T/K.T come out
    already in [d, s] layout, no flip transposes. W columns are permuted
    on the host so each head's rows are [evens(32) | odds(32)] (RoPE
    pairs de-interleaved; scores are invariant to a shared d-permutation
    of Q and K).
  - RoPE on DVE in [d, s] layout: partner rows fetched with 32-partition
    cross-quadrant copies, then out = ppt*cos + partner*sin(signed) as
    three [128, 512] tensor_tensor ops fused with the PSUM->SBUF
    eviction (V rows pass through via cos=1/sin=0 table entries).
  - V.T flipped back to [kv, d] via 4 small PE transposes per s-tile.
  - Attention in transposed layout: S.T[kv,q] = K @ Q.T per 128-kv block,
    exp on ACT (scale=1/8 folded in) with fp16 output, causal handled by
    skipping blocks above the diagonal + multiplying the diagonal block
    of P by a 0/1 fp16 mask, ctx.T[65,q] = [V|1].T @ P.T accumulated in
    fp32 PSUM (row 64 = softmax sums).
  - Finalize: 4 PE transposes of ctx.T into one [128, 4x66] PSUM tile
    (shared with the cxt pool), one reciprocal, 4 scalar muls, one DMA
    per (head, s-tile).
"""

import numpy as np
from contextlib import ExitStack

import concourse.bass as bass
import concourse.bacc as bacc
import concourse.mybir as mybir
from concourse import tile
from concourse.bass_utils import run_bass_kernel_spmd

F32 = mybir.dt.float32
F16 = mybir.dt.float16
MUL = mybir.AluOpType.mult
ADD = mybir.AluOpType.add

B = 2
S = 2048
DIN = 2048
D = 64              # head dim
HPC = 4             # query heads per core
NCORES = 8
WCOLS = 4 * D + D + D  # 256 q cols + 64 k + 64 v = 384
ST = 512            # s-tile (rows per outer step)
NST = B * S // ST   # 8 s-tiles
NCH = DIN // 128    # 16 k-chunks
NKV = S // 128      # kv tiles per batch


def build_bass():
    nc = bacc.Bacc(None, target_bir_lowering=False)
    xt_d = nc.declare_dram_parameter("xt", [DIN, B * S], F16, isOutput=False)
    w_d = nc.declare_dram_parameter("w", [DIN, WCOLS], F16, isOutput=False)
    cq_d = nc.declare_dram_parameter("cq", [128, S], F16, isOutput=False)
    sq_d = nc.declare_dram_parameter("sq", [128, S], F16, isOutput=False)
    ck_d = nc.declare_dram_parameter("ck", [128, S], F16, isOutput=False)
    sk_d = nc.declare_dram_parameter("sk", [128, S], F16, isOutput=False)
    mask_d = nc.declare_dram_parameter("mask", [128, 128], F16, isOutput=False)
    idlo_d = nc.declare_dram_parameter("idlo", [128, 64], F16, isOutput=False)
    id32_d = nc.declare_dram_parameter("id32", [128, 128], F32, isOutput=False)
    out_d = nc.declare_dram_parameter("out", [B * S, HPC * D], F32, isOutput=True)

    with ExitStack() as ctx:
        tc = ctx.enter_context(tile.TileContext(nc))
        const = ctx.enter_context(tc.tile_pool(name="const", bufs=1))
        resid = ctx.enter_context(tc.tile_pool(name="resid", bufs=1))
        xt_p = ctx.enter_context(tc.tile_pool(name="xt", bufs=2))
        ro_p = ctx.enter_context(tc.tile_pool(name="ro", bufs=2))
        qt_p = ctx.enter_context(tc.tile_pool(name="qt", bufs=4))
        kvq_p = ctx.enter_context(tc.tile_pool(name="kvq", bufs=2))
        p_p = ctx.enter_context(tc.tile_pool(name="p", bufs=3))
        cx_p = ctx.enter_context(tc.tile_pool(name="cx", bufs=2))
        o_p = ctx.enter_context(tc.tile_pool(name="o", bufs=3))
        rv_p = ctx.enter_context(tc.tile_pool(name="rv", bufs=3))
        tp_ps = ctx.enter_context(tc.tile_pool(name="tp_ps", bufs=2, space="PSUM"))
        pr_ps = ctx.enter_context(tc.tile_pool(name="pr_ps", bufs=2, space="PSUM"))
        sc_ps = ctx.enter_context(tc.tile_pool(name="sc_ps", bufs=2, space="PSUM"))
        cx_ps = ctx.enter_context(tc.tile_pool(name="cx_ps", bufs=2, space="PSUM"))

        # constants on the SCALAR engine's DMA queue so the x.T tiles (sync
        # queue) stream in parallel — the first projection only waits for
        # w chunk-group 0 + xt chunk-group 0 instead of ~7MB of constants.
        w_sb = const.tile([128, NCH, WCOLS], F16)
        for cg in range(4):
            nc.scalar.dma_start(
                out=w_sb[:, cg * 4:(cg + 1) * 4, :],
                in_=w_d[cg * 512:(cg + 1) * 512, :].rearrange(
                    "(c p) n -> p c n", p=128))
        mask_sb = const.tile([128, 128], F16)
        nc.scalar.dma_start(out=mask_sb[:], in_=mask_d[:])
        idlo = const.tile([128, 64], F16)
        nc.scalar.dma_start(out=idlo[:], in_=idlo_d[:])
        id32 = const.tile([128, 128], F32)
        nc.scalar.dma_start(out=id32[:], in_=id32_d[:])
        # rope tables in [d-row, s] layout, fp16; ~4KB/partition each
        cq = const.tile([128, S], F16)
        nc.scalar.dma_start(out=cq[:], in_=cq_d[:])
        sq = const.tile([128, S], F16)
        nc.scalar.dma_start(out=sq[:], in_=sq_d[:])
        ck = const.tile([128, S], F16)
        nc.scalar.dma_start(out=ck[:], in_=ck_d[:])
        sk = const.tile([128, S], F16)
        nc.scalar.dma_start(out=sk[:], in_=sk_d[:])

        # rows 0-63: K.T (RoPE'd); rows 64-127: duplicate copy so that the
        # scores matmul lhsT can match either base partition of the Q halves
        kt_res = resid.tile([128, B * S], F16)
        vp_res = resid.tile([128, B * NKV, 128], F16)  # [V|1|0pad] kv-tiles
        nc.vector.memset(vp_res[:], 0.0)
        nc.vector.memset(vp_res[:, :, 64:65], 1.0)

        for st in range(NST):
            b, sti = divmod(st, 4)
            ssl = slice(sti * ST, (sti + 1) * ST)  # within-batch s range

            # ---- x.T tile straight from HBM (host-transposed), split so the
            # first chunk-group's projections can start before the rest land
            xt = xt_p.tile([128, NCH, ST], F16, tag="xt")
            for cg in range(4):
                nc.sync.dma_start(
                    out=xt[:, cg * 4:(cg + 1) * 4, :],
                    in_=xt_d[cg * 512:(cg + 1) * 512,
                             st * ST:(st + 1) * ST].rearrange(
                                 "(c p) s -> p c s", p=128))

            # ---- W-stationary projections + RoPE in [d, s] layout ----
            # wb 0: [h0_e h0_o h1_e h1_o], wb 1: [h2_e h2_o h3_e h3_o],
            # wb 2: [k_e k_o | V(64 natural cols, transposed layout)]
            qta = qt_p.tile([128, ST], F16, tag="qta")
            qtb = qt_p.tile([128, ST], F16, tag="qtb")
            kvq = kvq_p.tile([128, ST], F16, tag="kvq")
            for wb in range(3):
                ppt = pr_ps.tile([128, ST], F32, tag="ppt")
                for c in range(NCH):
                    nc.tensor.matmul(
                        ppt[:], w_sb[:, c, wb * 128:(wb + 1) * 128],
                        xt[:, c, :], start=(c == 0), stop=(c == NCH - 1))
                # partner rows for the rotation (32-part cross-quadrant
                # copies), then qdst = ppt*C + partner*S_signed
                sh = ro_p.tile([128, ST], F32, tag="sh")
                nc.vector.tensor_copy(sh[0:32, :], ppt[32:64, :])
                nc.vector.tensor_copy(sh[32:64, :], ppt[0:32, :])
                if wb < 2:
                    nc.vector.tensor_copy(sh[64:96, :], ppt[96:128, :])
                    nc.vector.tensor_copy(sh[96:128, :], ppt[64:96, :])
                    qdst, tc_, ts_ = (qta if wb == 0 else qtb), cq, sq
                else:
                    # V rows pass through (ck=1, sk=0); keep sh initialized
                    nc.vector.tensor_copy(sh[64:128, :], ppt[64:128, :])
                    qdst, tc_, ts_ = kvq, ck, sk
                ts2 = ro_p.tile([128, ST], F32, tag="ts2")
                nc.vector.tensor_tensor(qdst[:], ppt[:], tc_[:, ssl], MUL)
                nc.vector.tensor_tensor(ts2[:], sh[:], ts_[:, ssl], MUL)
                nc.vector.tensor_tensor(qdst[:], qdst[:], ts2[:], ADD)
            # K.T rows into the resident (plus base-64 duplicate via DMA)
            nc.vector.tensor_copy(
                kt_res[0:64, st * ST:(st + 1) * ST], kvq[0:64, :])
            nc.sync.dma_start(
                out=kt_res[64:128, st * ST:(st + 1) * ST],
                in_=kt_res[0:64, st * ST:(st + 1) * ST])
            # V.T -> [kv, d] natural via small PE transposes
            for vc in range(4):
                tpv = tp_ps.tile([128, 64], F16, tag="tp")
                nc.tensor.transpose(
                    tpv[:], kvq[64:128, vc * 128:(vc + 1) * 128],
                    idlo[64:128, :])
                nc.vector.tensor_copy(
                    vp_res[:, b * NKV + sti * 4 + vc, 0:64], tpv[:])

            # ---- attention for the 4 heads of this q-tile ----
            js = [4 * sti] + list(range(4 * sti)) + \
                 [4 * sti + 1, 4 * sti + 2, 4 * sti + 3]
            for h in range(HPC):
                p0 = (h % 2) * 64
                qh = (qta if h < 2 else qtb)[p0:p0 + 64, :]
                cxt = cx_ps.tile([128, ST], F32, tag="cxt")
                for idx, j in enumerate(js):
                    off = 128 * j - 512 * sti
                    w0 = max(0, off)
                    sc = sc_ps.tile([128, ST], F32, tag="sc")
                    nc.tensor.matmul(
                        sc[:, w0:ST],
                        kt_res[p0:p0 + 64, b * S + j * 128:b * S + (j + 1) * 128],
                        qh[:, w0:ST], start=True, stop=True)
                    psb = p_p.tile([128, ST], F16, tag="psb")
                    nc.scalar.activation(
                        psb[:, w0:ST], sc[:, w0:ST],
                        mybir.ActivationFunctionType.Exp, scale=0.125)
                    if j >= 4 * sti:
                        # zero the upper-triangle of the diagonal block
                        nc.vector.tensor_tensor(
                            psb[:, off:off + 128], psb[:, off:off + 128],
                            mask_sb[:], MUL)
                    nc.tensor.matmul(
                        cxt[:, w0:ST], vp_res[:, b * NKV + j, :],
                        psb[:, w0:ST],
                        start=(idx == 0), stop=(idx == len(js) - 1))
                cxs = cx_p.tile([65, ST], F32, tag="cxs")
                nc.vector.tensor_copy(cxs[:], cxt[0:65, :])
                # fi shares the cx_ps buffers (same tag/shape as cxt)
                fi = cx_ps.tile([128, ST], F32, tag="cxt")
                for qq in range(4):
                    nc.tensor.transpose(
                        fi[:, qq * 128:qq * 128 + 66],
                        cxs[:, qq * 128:(qq + 1) * 128],
                        id32[0:65, 0:66])
                rv = rv_p.tile([128, 4], F32, tag="rv")
                nc.vector.reciprocal(rv[:], fi[:, 64:ST:128])
                ob = o_p.tile([128, 4, 64], F32, tag="ob")
                for qq in range(4):
                    nc.vector.tensor_scalar_mul(
                        ob[:, qq, :], fi[:, qq * 128:qq * 128 + 64],
                        rv[:, qq:qq + 1])
                nc.sync.dma_start(
                    out=out_d[st * ST:(st + 1) * ST,
                              h * 64:(h + 1) * 64].rearrange(
                                  "(q p) d -> p q d", p=128),
                    in_=ob[:])
    return nc


_NC_CACHE = None


def _host_consts():
    i = np.arange(0, D, 2, dtype=np.float64) / D          # 32 pair exponents
    freqs = 1.0 / (10000.0 ** i)                           # (32,)
    ang = np.arange(S, dtype=np.float64)[:, None] * freqs[None, :]  # (S, 32)
    cos32 = np.cos(ang).astype(np.float32).T               # (32, S)
    sin32 = np.sin(ang).astype(np.float32).T
    ones = np.ones((64, S), np.float32)
    zeros = np.zeros((64, S), np.float32)
    cq = np.vstack([cos32, cos32, cos32, cos32]).astype(np.float16)
    sq = np.vstack([-sin32, sin32, -sin32, sin32]).astype(np.float16)
    ck = np.vstack([cos32, cos32, ones]).astype(np.float16)
    sk = np.vstack([-sin32, sin32, zeros]).astype(np.float16)
    kv, qq = np.meshgrid(np.arange(128), np.arange(128), indexing="ij")
    mask01 = (kv <= qq).astype(np.float16)                 # 1 = allowed
    idlo = np.zeros((128, 64), np.float16)
    idlo[64:128] = np.eye(64, dtype=np.float16)
    ident32 = np.eye(128, dtype=np.float32)
    return cq, sq, ck, sk, mask01, idlo, ident32


def _deint(w):
    # de-interleave rope pairs per 64-col head: [evens | odds]
    return np.hstack([w[:, 0::2], w[:, 1::2]])


def _in_maps(x, Wq, Wk, Wv):
    x = np.asarray(x, dtype=np.float32).reshape(B * S, DIN)
    xt = np.ascontiguousarray(x.T).astype(np.float16)      # [DIN, B*S]
    Wq = np.asarray(Wq, dtype=np.float32)
    Wk = np.asarray(Wk, dtype=np.float32)
    Wv = np.asarray(Wv, dtype=np.float32)
    cq, sq, ck, sk, mask01, idlo, ident32 = _host_consts()

    in_maps = []
    for k in range(NCORES):
        cols = []
        for h in range(4):
            cols.append(_deint(Wq[:, (4 * k + h) * 64:(4 * k + h + 1) * 64]))
        cols.append(_deint(Wk[:, k * 64:(k + 1) * 64]))
        cols.append(Wv[:, k * 64:(k + 1) * 64])
        w_all = np.hstack(cols).astype(np.float16)
        in_maps.append({
            "xt": xt, "w": np.ascontiguousarray(w_all),
            "cq": cq, "sq": sq, "ck": ck, "sk": sk, "mask": mask01,
            "idlo": idlo, "id32": ident32,
        })
    return in_maps


def _run(in_maps, **kwargs):
    global _NC_CACHE
    if _NC_CACHE is None:
        _NC_CACHE = build_bass()
        _NC_CACHE.finalize()
    return run_bass_kernel_spmd(_NC_CACHE, in_maps, list(range(NCORES)),
                                **kwargs)


def kernel(x, Wq, Wk, Wv):
    res = _run(_in_maps(x, Wq, Wk, Wv))
    out = np.concatenate([res.results[k]["out"] for k in range(NCORES)], axis=1)
    return out.reshape(B, S, 32 * D)


# revision 19
# speedup vs baseline: 1.3810x; 1.0140x over previous
"""GQA forward (b=2, s=2048, H=32 q heads, 8 kv heads, d=64) on 8 TRN2 cores.

Sharding: core k owns query heads 4k..4k+3 and kv head k. GQA group
structure makes attention fully local per core (q heads 4k..4k+3 attend
only to kv head k). x is replicated; W columns are sharded; outputs are
column-concatenated.

Per-core kernel (Tile framework), fp16 datapath / fp32 accumulation:
  - x.T is produced on the HOST (numpy transpose + fp16 cast) and DMA'd
    straight into SBUF — no on-chip transposes of x.
  - W-stationary projections: QKV.T[wcol, s] = W_chunk.T @ xT accumulated
    in fp32 PSUM over 16 k-chunks, 512-wide moving — Q.T/K.T come out
    already in [d, s] layout, no flip transposes. W columns are permuted
    on the host so each head's rows are [evens(32) | odds(32)] (RoPE
    pairs de-interleaved; scores are invariant to a shared d-permutation
    of Q and K).
  - RoPE on DVE in [d, s] layout: partner rows fetched with 32-partition
    cross-quadrant copies, then out = ppt*cos + partner*sin(signed),
    fused with the PSUM->SBUF eviction (V rows pass through via
    cos=1/sin=0 table entries). V.T flipped back to [kv, d] on the PE.
  - Attention in transposed layout: S.T[kv,q] = K @ Q.T per 128-kv block,
    exp on ACT (scale=1/8 folded in) with fp16 output, causal handled by
    skipping blocks above the diagonal + multiplying the diagonal block
    of P by a 0/1 fp16 mask, ctx.T[65,q] = [V|1].T @ P.T accumulated in
    fp32 PSUM (row 64 = softmax sums).
  - SOFTWARE PIPELINING: the attention phase is paced by the ACT exp
    (622ns/block vs ~426ns of PE work), and the PE executes in program
    order — so projection instructions for s-tile st+1 are WOVEN between
    attention units of s-tile st, and each ctx matmul is emitted one
    unit after its scores matmul, keeping the PE fed during exp waits.
  - Finalize: 4 PE transposes of ctx.T into a PSUM tile shared with the
    cxt pool, one reciprocal, 4 scalar muls, one DMA per (head, s-tile).
"""

import numpy as np
from contextlib import ExitStack

import concourse.bass as bass
import concourse.bacc as bacc
import concourse.mybir as mybir
from concourse import tile
from concourse.bass_utils import run_bass_kernel_spmd

F32 = mybir.dt.float32
F16 = mybir.dt.float16
MUL = mybir.AluOpType.mult
ADD = mybir.AluOpType.add

B = 2
S = 2048
DIN = 2048
D = 64              # head dim
HPC = 4             # query heads per core
NCORES = 8
WCOLS = 4 * D + D + D  # 256 q cols + 64 k + 64 v = 384
ST = 512            # s-tile (rows per outer step)
NST = B * S // ST   # 8 s-tiles
NCH = DIN // 128    # 16 k-chunks
NKV = S // 128      # kv tiles per batch


def build_bass():
    nc = bacc.Bacc(None, target_bir_lowering=False)
    xt_d = nc.declare_dram_parameter("xt", [DIN, B * S], F16, isOutput=False)
    w_d = nc.declare_dram_parameter("w", [DIN, WCOLS], F16, isOutput=False)
    cq_d = nc.declare_dram_parameter("cq", [128, S], F16, isOutput=False)
    sq_d = nc.declare_dram_parameter("sq", [128, S], F16, isOutput=False)
    ck_d = nc.declare_dram_parameter("ck", [128, S], F16, isOutput=False)
    sk_d = nc.declare_dram_parameter("sk", [128, S], F16, isOutput=False)
    mask_d = nc.declare_dram_parameter("mask", [128, 128], F16, isOutput=False)
    idlo_d = nc.declare_dram_parameter("idlo", [128, 64], F16, isOutput=False)
    id32_d = nc.declare_dram_parameter("id32", [128, 128], F32, isOutput=False)
    out_d = nc.declare_dram_parameter("out", [B * S, HPC * D], F32, isOutput=True)

    with ExitStack() as ctx:
        tc = ctx.enter_context(tile.TileContext(nc))
        const = ctx.enter_context(tc.tile_pool(name="const", bufs=1))
        resid = ctx.enter_context(tc.tile_pool(name="resid", bufs=1))
        xt_p = ctx.enter_context(tc.tile_pool(name="xt", bufs=2))
        ro_p = ctx.enter_context(tc.tile_pool(name="ro", bufs=2))
        qt_p = ctx.enter_context(tc.tile_pool(name="qt", bufs=3))
        kvq_p = ctx.enter_context(tc.tile_pool(name="kvq", bufs=2))
        p_p = ctx.enter_context(tc.tile_pool(name="p", bufs=3))
        cx_p = ctx.enter_context(tc.tile_pool(name="cx", bufs=2))
        o_p = ctx.enter_context(tc.tile_pool(name="o", bufs=3))
        rv_p = ctx.enter_context(tc.tile_pool(name="rv", bufs=3))
        tp_ps = ctx.enter_context(tc.tile_pool(name="tp_ps", bufs=2, space="PSUM"))
        pr_ps = ctx.enter_context(tc.tile_pool(name="pr_ps", bufs=2, space="PSUM"))
        sc_ps = ctx.enter_context(tc.tile_pool(name="sc_ps", bufs=2, space="PSUM"))
        cx_ps = ctx.enter_context(tc.tile_pool(name="cx_ps", bufs=2, space="PSUM"))

        # constants on the SCALAR engine's DMA queue so the x.T tiles (sync
        # queue) stream in parallel — the first projection only waits for
        # w chunk-group 0 + xt chunk-group 0 instead of ~7MB of constants.
        w_sb = const.tile([128, NCH, WCOLS], F16)
        for cg in range(4):
            nc.scalar.dma_start(
                out=w_sb[:, cg * 4:(cg + 1) * 4, :],
                in_=w_d[cg * 512:(cg + 1) * 512, :].rearrange(
                    "(c p) n -> p c n", p=128))
        mask_sb = const.tile([128, 128], F16)
        nc.scalar.dma_start(out=mask_sb[:], in_=mask_d[:])
        idlo = const.tile([128, 64], F16)
        nc.scalar.dma_start(out=idlo[:], in_=idlo_d[:])
        id32 = const.tile([128, 128], F32)
        nc.scalar.dma_start(out=id32[:], in_=id32_d[:])
        # rope tables in [d-row, s] layout, fp16; ~4KB/partition each
        cq = const.tile([128, S], F16)
        nc.scalar.dma_start(out=cq[:], in_=cq_d[:])
        sq = const.tile([128, S], F16)
        nc.scalar.dma_start(out=sq[:], in_=sq_d[:])
        ck = const.tile([128, S], F16)
        nc.scalar.dma_start(out=ck[:], in_=ck_d[:])
        sk = const.tile([128, S], F16)
        nc.scalar.dma_start(out=sk[:], in_=sk_d[:])

        # rows 0-63: K.T (RoPE'd); rows 64-127: duplicate copy so that the
        # scores matmul lhsT can match either base partition of the Q halves
        kt_res = resid.tile([128, B * S], F16)
        vp_res = resid.tile([128, B * NKV, 128], F16)  # [V|1|0pad] kv-tiles
        nc.vector.memset(vp_res[:], 0.0)
        nc.vector.memset(vp_res[:, :, 64:65], 1.0)

        qts = {}  # st -> (qta, qtb) for the attention generator

        def proj_units(st):
            """Generator emitting projection+RoPE for s-tile st, yielding
            after each instruction so it can be woven into attention."""
            b, sti = divmod(st, 4)
            ssl = slice(sti * ST, (sti + 1) * ST)
            xt = xt_p.tile([128, NCH, ST], F16, tag="xt")
            for cg in range(4):
                nc.sync.dma_start(
                    out=xt[:, cg * 4:(cg + 1) * 4, :],
                    in_=xt_d[cg * 512:(cg + 1) * 512,
                             st * ST:(st + 1) * ST].rearrange(
                                 "(c p) s -> p c s", p=128))
            qta = qt_p.tile([128, ST], F16, tag="qta")
            qtb = qt_p.tile([128, ST], F16, tag="qtb")
            kvq = kvq_p.tile([128, ST], F16, tag="kvq")
            qts[st] = (qta, qtb)
            for wb in range(3):
                ppt = pr_ps.tile([128, ST], F32, tag="ppt")
                for c in range(NCH):
                    nc.tensor.matmul(
                        ppt[:], w_sb[:, c, wb * 128:(wb + 1) * 128],
                        xt[:, c, :], start=(c == 0), stop=(c == NCH - 1))
                    yield
                sh = ro_p.tile([128, ST], F32, tag="sh")
                nc.vector.tensor_copy(sh[0:32, :], ppt[32:64, :])
                nc.vector.tensor_copy(sh[32:64, :], ppt[0:32, :])
                yield
                if wb < 2:
                    nc.vector.tensor_copy(sh[64:96, :], ppt[96:128, :])
                    nc.vector.tensor_copy(sh[96:128, :], ppt[64:96, :])
                    qdst, tc_, ts_ = (qta if wb == 0 else qtb), cq, sq
                else:
                    nc.vector.tensor_copy(sh[64:128, :], ppt[64:128, :])
                    qdst, tc_, ts_ = kvq, ck, sk
                yield
                ts2 = ro_p.tile([128, ST], F32, tag="ts2")
                nc.vector.tensor_tensor(qdst[:], ppt[:], tc_[:, ssl], MUL)
                yield
                nc.vector.tensor_tensor(ts2[:], sh[:], ts_[:, ssl], MUL)
                yield
                nc.vector.tensor_tensor(qdst[:], qdst[:], ts2[:], ADD)
                yield
            nc.vector.tensor_copy(
                kt_res[0:64, st * ST:(st + 1) * ST], kvq[0:64, :])
            nc.sync.dma_start(
                out=kt_res[64:128, st * ST:(st + 1) * ST],
                in_=kt_res[0:64, st * ST:(st + 1) * ST])
            yield
            for vc in range(4):
                tpv = tp_ps.tile([128, 64], F16, tag="tp")
                nc.tensor.transpose(
                    tpv[:], kvq[64:128, vc * 128:(vc + 1) * 128],
                    idlo[64:128, :])
                nc.vector.tensor_copy(
                    vp_res[:, b * NKV + sti * 4 + vc, 0:64], tpv[:])
                yield

        def attn_units(st):
            """Generator emitting attention for s-tile st, one (scores, exp)
            step per yield; each ctx matmul is emitted one step behind its
            scores so the PE isn't blocked on the exp."""
            b, sti = divmod(st, 4)
            js = [4 * sti] + list(range(4 * sti)) + \
                 [4 * sti + 1, 4 * sti + 2, 4 * sti + 3]
            qta, qtb = qts.pop(st)
            for h in range(HPC):
                p0 = (h % 2) * 64
                qh = (qta if h < 2 else qtb)[p0:p0 + 64, :]
                cxt = cx_ps.tile([128, ST], F32, tag="cxt")
                pend = None  # (psb, w0, start, slot) awaiting its ctx matmul
                for idx, j in enumerate(js):
                    off = 128 * j - 512 * sti
                    w0 = max(0, off)
                    sc = sc_ps.tile([128, ST], F32, tag="sc")
                    nc.tensor.matmul(
                        sc[:, w0:ST],
                        kt_res[p0:p0 + 64,
                               b * S + j * 128:b * S + (j + 1) * 128],
                        qh[:, w0:ST], start=True, stop=True)
                    psb = p_p.tile([128, ST], F16, tag="psb")
                    nc.scalar.activation(
                        psb[:, w0:ST], sc[:, w0:ST],
                        mybir.ActivationFunctionType.Exp, scale=0.125)
                    if j >= 4 * sti:
                        nc.vector.tensor_tensor(
                            psb[:, off:off + 128], psb[:, off:off + 128],
                            mask_sb[:], MUL)
                    if pend is not None:
                        nc.tensor.matmul(
                            cxt[:, pend[1]:ST], vp_res[:, pend[3], :],
                            pend[0][:, pend[1]:ST],
                            start=pend[2], stop=False)
                    pend = (psb, w0, idx == 0, b * NKV + j)
                    yield
                nc.tensor.matmul(
                    cxt[:, pend[1]:ST], vp_res[:, pend[3], :],
                    pend[0][:, pend[1]:ST], start=pend[2], stop=True)
                cxs = cx_p.tile([65, ST], F32, tag="cxs")
                nc.vector.tensor_copy(cxs[:], cxt[0:65, :])
                yield
                fi = cx_ps.tile([128, ST], F32, tag="cxt")
                for qq in range(4):
                    nc.tensor.transpose(
                        fi[:, qq * 128:qq * 128 + 66],
                        cxs[:, qq * 128:(qq + 1) * 128],
                        id32[0:65, 0:66])
                rv = rv_p.tile([128, 4], F32, tag="rv")
                nc.vector.reciprocal(rv[:], fi[:, 64:ST:128])
                ob = o_p.tile([128, 4, 64], F32, tag="ob")
                for qq in range(4):
                    nc.vector.tensor_scalar_mul(
                        ob[:, qq, :], fi[:, qq * 128:qq * 128 + 64],
                        rv[:, qq:qq + 1])
                nc.sync.dma_start(
                    out=out_d[st * ST:(st + 1) * ST,
                              h * 64:(h + 1) * 64].rearrange(
                                  "(q p) d -> p q d", p=128),
                    in_=ob[:])
                yield

        for st in range(NST + 1):
            pg = proj_units(st) if st < NST else None
            ag = attn_units(st - 1) if st > 0 else None
            if ag is None:
                for _ in pg:
                    pass
            else:
                for _ in ag:
                    if pg is not None:
                        if next(pg, StopIteration) is StopIteration or \
                                next(pg, StopIteration) is StopIteration:
                            pg = None
                if pg is not None:
                    for _ in pg:
                        pass
    return nc


_NC_CACHE = None


def _host_consts():
    i = np.arange(0, D, 2, dtype=np.float64) / D          # 32 pair exponents
    freqs = 1.0 / (10000.0 ** i)                           # (32,)
    ang = np.arange(S, dtype=np.float64)[:, None] * freqs[None, :]  # (S, 32)
    cos32 = np.cos(ang).astype(np.float32).T               # (32, S)
    sin32 = np.sin(ang).astype(np.float32).T
    ones = np.ones((64, S), np.float32)
    zeros = np.zeros((64, S), np.float32)
    cq = np.vstack([cos32, cos32, cos32, cos32]).astype(np.float16)
    sq = np.vstack([-sin32, sin32, -sin32, sin32]).astype(np.float16)
    ck = np.vstack([cos32, cos32, ones]).astype(np.float16)
    sk = np.vstack([-sin32, sin32, zeros]).astype(np.float16)
    kv, qq = np.meshgrid(np.arange(128), np.arange(128), indexing="ij")
    mask01 = (kv <= qq).astype(np.float16)                 # 1 = allowed
    idlo = np.zeros((128, 64), np.float16)
    idlo[64:128] = np.eye(64, dtype=np.float16)
    ident32 = np.eye(128, dtype=np.float32)
    return cq, sq, ck, sk, mask01, idlo, ident32


def _deint(w):
    # de-interleave rope pairs per 64-col head: [evens | odds]
    return np.hstack([w[:, 0::2], w[:, 1::2]])


def _in_maps(x, Wq, Wk, Wv):
    x = np.asarray(x, dtype=np.float32).reshape(B * S, DIN)
    xt = np.ascontiguousarray(x.T).astype(np.float16)      # [DIN, B*S]
    Wq = np.asarray(Wq, dtype=np.float32)
    Wk = np.asarray(Wk, dtype=np.float32)
    Wv = np.asarray(Wv, dtype=np.float32)
    cq, sq, ck, sk, mask01, idlo, ident32 = _host_consts()

    in_maps = []
    for k in range(NCORES):
        cols = []
        for h in range(4):
            cols.append(_deint(Wq[:, (4 * k + h) * 64:(4 * k + h + 1) * 64]))
        cols.append(_deint(Wk[:, k * 64:(k + 1) * 64]))
        cols.append(Wv[:, k * 64:(k + 1) * 64])
        w_all = np.hstack(cols).astype(np.float16)
        in_maps.append({
            "xt": xt, "w": np.ascontiguousarray(w_all),
            "cq": cq, "sq": sq, "ck": ck, "sk": sk, "mask": mask01,
            "idlo": idlo, "id32": ident32,
        })
    return in_maps


def _run(in_maps, **kwargs):
    global _NC_CACHE
    if _NC_CACHE is None:
        _NC_CACHE = build_bass()
        _NC_CACHE.finalize()
    return run_bass_kernel_spmd(_NC_CACHE, in_maps, list(range(NCORES)),
                                **kwargs)


def kernel(x, Wq, Wk, Wv):
    res = _run(_in_maps(x, Wq, Wk, Wv))
    out = np.concatenate([res.results[k]["out"] for k in range(NCORES)], axis=1)
    return out.reshape(B, S, 32 * D)


# revision 22
# speedup vs baseline: 1.4048x; 1.0172x over previous
"""GQA forward (b=2, s=2048, H=32 q heads, 8 kv heads, d=64) on 8 TRN2 cores.

Sharding: core k owns query heads 4k..4k+3 and kv head k. GQA group
structure makes attention fully local per core (q heads 4k..4k+3 attend
only to kv head k). x is replicated; W columns are sharded; outputs are
column-concatenated.

Per-core kernel (Tile framework), fp16 datapath / fp32 accumulation:
  - x.T is produced on the HOST (numpy transpose + fp16 cast) and DMA'd
    straight into SBUF — no on-chip transposes of x.
  - Projections in natural layout: QKV[s,384] = xT_chunk.T @ W_chunk
    accumulated in fp32 PSUM over 16 k-chunks (fp16 operands, 1 cyc/row).
  - RoPE on DVE with free-dim stride-2 views, fused with the PSUM->SBUF
    eviction (sin table pre-negated on host so plain tensor_tensor
    suffices); V columns go straight into the [V|1] resident.
  - Q/K flipped to [d, s] via PE transposes (fp16, 1 cyc/row).
  - Attention in transposed layout: S.T[kv,q] = K @ Q.T per 128-kv block,
    exp on ACT (scale=1/8 folded in) with fp16 output, causal handled by
    skipping blocks above the diagonal + multiplying the diagonal block
    of P by a 0/1 fp16 mask, ctx.T[65,q] = [V|1].T @ P.T accumulated in
    fp32 PSUM (row 64 = softmax sums).
  - Finalize: 4 PE transposes of ctx.T into one [128,4,66] PSUM bank,
    one reciprocal, 4 scalar muls, one DMA per (head, s-tile).
"""

import numpy as np
from contextlib import ExitStack

import concourse.bass as bass
import concourse.bacc as bacc
import concourse.mybir as mybir
from concourse import tile
from concourse.bass_utils import run_bass_kernel_spmd

F32 = mybir.dt.float32
F16 = mybir.dt.float16
MUL = mybir.AluOpType.mult
ADD = mybir.AluOpType.add

B = 2
S = 2048
DIN = 2048
D = 64              # head dim
HPC = 4             # query heads per core
NCORES = 8
WCOLS = 4 * D + D + D  # 256 q cols + 64 k + 64 v = 384
RC = 320            # roped columns (4 q heads + k head)
ST = 512            # s-tile (rows per outer step)
NST = B * S // ST   # 8 s-tiles
NCH = DIN // 128    # 16 k-chunks
NKV = S // 128      # kv tiles per batch


def build_bass():
    nc = bacc.Bacc(None, target_bir_lowering=False)
    xt_d = nc.declare_dram_parameter("xt", [DIN, B * S], F16, isOutput=False)
    w_d = nc.declare_dram_parameter("w", [DIN, WCOLS], F16, isOutput=False)
    cos_d = nc.declare_dram_parameter("cosn", [S, RC], F16, isOutput=False)
    sin_d = nc.declare_dram_parameter("sinn", [S, RC], F16, isOutput=False)
    mask_d = nc.declare_dram_parameter("mask", [128, 128], F16, isOutput=False)
    id16_d = nc.declare_dram_parameter("id16", [128, 128], F16, isOutput=False)
    id32_d = nc.declare_dram_parameter("id32", [128, 128], F32, isOutput=False)
    out_d = nc.declare_dram_parameter("out", [B * S, HPC * D], F32, isOutput=True)

    with ExitStack() as ctx:
        tc = ctx.enter_context(tile.TileContext(nc))
        const = ctx.enter_context(tc.tile_pool(name="const", bufs=1))
        resid = ctx.enter_context(tc.tile_pool(name="resid", bufs=1))
        xt_p = ctx.enter_context(tc.tile_pool(name="xt", bufs=2))
        qn_p = ctx.enter_context(tc.tile_pool(name="qn", bufs=3))
        qt_p = ctx.enter_context(tc.tile_pool(name="qt", bufs=4))
        p_p = ctx.enter_context(tc.tile_pool(name="p", bufs=3))
        cx_p = ctx.enter_context(tc.tile_pool(name="cx", bufs=2))
        o_p = ctx.enter_context(tc.tile_pool(name="o", bufs=3))
        rv_p = ctx.enter_context(tc.tile_pool(name="rv", bufs=3))
        tp_ps = ctx.enter_context(tc.tile_pool(name="tp_ps", bufs=2, space="PSUM"))
        pr_ps = ctx.enter_context(tc.tile_pool(name="pr_ps", bufs=2, space="PSUM"))
        sc_ps = ctx.enter_context(tc.tile_pool(name="sc_ps", bufs=2, space="PSUM"))
        cx_ps = ctx.enter_context(tc.tile_pool(name="cx_ps", bufs=2, space="PSUM"))

        # constants on the SCALAR engine's DMA queue so the x.T tiles (sync
        # queue) stream in parallel — the first projection only waits for
        # w chunk-group 0 + xt chunk-group 0 instead of ~7MB of constants.
        w_sb = const.tile([128, NCH, WCOLS], F16)
        for cg in range(4):
            nc.scalar.dma_start(
                out=w_sb[:, cg * 4:(cg + 1) * 4, :],
                in_=w_d[cg * 512:(cg + 1) * 512, :].rearrange(
                    "(c p) n -> p c n", p=128))
        mask_sb = const.tile([128, 128], F16)
        nc.scalar.dma_start(out=mask_sb[:], in_=mask_d[:])
        id16 = const.tile([128, 128], F16)
        nc.scalar.dma_start(out=id16[:], in_=id16_d[:])
        id32 = const.tile([128, 128], F32)
        nc.scalar.dma_start(out=id32[:], in_=id32_d[:])
        # rope tables resident, t-major ([128, t, col]); both batches share.
        # chunk-group cg holds t=4cg..4cg+3, exactly what s-tile cg%4 needs.
        ctab = const.tile([128, NCH, RC], F16)
        stab = const.tile([128, NCH, RC], F16)
        for cg in range(4):
            nc.scalar.dma_start(
                out=ctab[:, cg * 4:(cg + 1) * 4, :],
                in_=cos_d[cg * 512:(cg + 1) * 512, :].rearrange(
                    "(t p) n -> p t n", p=128))
            nc.scalar.dma_start(
                out=stab[:, cg * 4:(cg + 1) * 4, :],
                in_=sin_d[cg * 512:(cg + 1) * 512, :].rearrange(
                    "(t p) n -> p t n", p=128))

        # rows 0-63: K.T (RoPE'd); rows 64-127: duplicate copy so that the
        # scores matmul lhsT can match either base partition of the Q halves
        kt_res = resid.tile([128, B * S], F16)
        vp_res = resid.tile([128, B * NKV, 128], F16)  # [V|1|0pad] kv-tiles
        nc.vector.memset(vp_res[:], 0.0)
        nc.vector.memset(vp_res[:, :, 64:65], 1.0)

        for st in range(NST):
            b, sti = divmod(st, 4)

            # ---- x.T tile straight from HBM (host-transposed), split so the
            # first chunk-group's projections can start before the rest land
            xt = xt_p.tile([128, NCH, ST], F16, tag="xt")
            for cg in range(4):
                nc.sync.dma_start(
                    out=xt[:, cg * 4:(cg + 1) * 4, :],
                    in_=xt_d[cg * 512:(cg + 1) * 512,
                             st * ST:(st + 1) * ST].rearrange(
                                 "(c p) s -> p c s", p=128))

            # ---- projections (natural layout) + RoPE + transposes ----
            qta = qt_p.tile([128, ST], F16, tag="qta")   # heads 0,1 as [d,s]
            qtb = qt_p.tile([128, ST], F16, tag="qtb")   # heads 2,3 as [d,s]
            for pt in range(4):
                t = sti * 4 + pt  # within-batch 128-row block index
                pp = pr_ps.tile([128, WCOLS], F32, tag="pp")
                for c in range(NCH):
                    nc.tensor.matmul(
                        pp[:], xt[:, c, pt * 128:(pt + 1) * 128],
                        w_sb[:, c, :], start=(c == 0), stop=(c == NCH - 1))
                qn = qn_p.tile([128, RC], F16, tag="qn")
                ts = qn_p.tile([128, RC], F32, tag="ts")
                # even cols: qe*c - qo*s ; odd cols: qo*c + qe*s
                # (sin table pre-negated on host in even columns)
                nc.vector.tensor_tensor(
                    ts[:, 0:RC:2], pp[:, 1:RC:2], stab[:, t, 0:RC:2], MUL)
                nc.vector.tensor_tensor(
                    ts[:, 1:RC:2], pp[:, 0:RC:2], stab[:, t, 1:RC:2], MUL)
                nc.vector.tensor_tensor(qn[:], pp[:, 0:RC], ctab[:, t, :], MUL)
                nc.vector.tensor_tensor(qn[:], qn[:], ts[:], ADD)
                # V columns: straight into the [V|1] resident ([kv, d] natural)
                nc.vector.tensor_copy(
                    vp_res[:, b * NKV + t, 0:64], pp[:, RC:WCOLS])
                # flip Q to [d, s]
                for cb in range(2):
                    tp = tp_ps.tile([128, 128], F16, tag="tp")
                    nc.tensor.transpose(
                        tp[:], qn[:, cb * 128:(cb + 1) * 128], id16[:])
                    dst = qta if cb == 0 else qtb
                    nc.vector.tensor_copy(
                        dst[:, pt * 128:(pt + 1) * 128], tp[:])
                # flip K ([128, 64] -> [64, 128])
                tpk = tp_ps.tile([128, 128], F16, tag="tp")
                nc.tensor.transpose(tpk[0:64, :], qn[:, 256:320], id16[:])
                nc.vector.tensor_copy(
                    kt_res[0:64, st * ST + pt * 128:st * ST + (pt + 1) * 128],
                    tpk[0:64, :])
            nc.sync.dma_start(
                out=kt_res[64:128, st * ST:(st + 1) * ST],
                in_=kt_res[0:64, st * ST:(st + 1) * ST])

            # ---- attention for the 4 heads of this q-tile ----
            js = [4 * sti] + list(range(4 * sti)) + \
                 [4 * sti + 1, 4 * sti + 2, 4 * sti + 3]
            for h in range(HPC):
                p0 = (h % 2) * 64
                qh = (qta if h < 2 else qtb)[p0:p0 + 64, :]
                cxt = cx_ps.tile([128, ST], F32, tag="cxt")
                for idx, j in enumerate(js):
                    off = 128 * j - 512 * sti
                    w0 = max(0, off)
                    sc = sc_ps.tile([128, ST], F32, tag="sc")
                    nc.tensor.matmul(
                        sc[:, w0:ST],
                        kt_res[p0:p0 + 64, b * S + j * 128:b * S + (j + 1) * 128],
                        qh[:, w0:ST], start=True, stop=True)
                    psb = p_p.tile([128, ST], F16, tag="psb")
                    nc.scalar.activation(
                        psb[:, w0:ST], sc[:, w0:ST],
                        mybir.ActivationFunctionType.Exp, scale=0.125)
                    if j >= 4 * sti:
                        # zero the upper-triangle of the diagonal block
                        nc.vector.tensor_tensor(
                            psb[:, off:off + 128], psb[:, off:off + 128],
                            mask_sb[:], MUL)
                    nc.tensor.matmul(
                        cxt[:, w0:ST], vp_res[:, b * NKV + j, :],
                        psb[:, w0:ST],
                        start=(idx == 0), stop=(idx == len(js) - 1))
                cxs = cx_p.tile([65, ST], F32, tag="cxs")
                nc.vector.tensor_copy(cxs[:], cxt[0:65, :])
                # fi shares the cx_ps buffers (same tag/shape as cxt): ctx of
                # head h+1 reuses the buffer fi of head h-1 released
                fi = cx_ps.tile([128, ST], F32, tag="cxt")
                for qq in range(4):
                    nc.tensor.transpose(
                        fi[:, qq * 128:qq * 128 + 66],
                        cxs[:, qq * 128:(qq + 1) * 128],
                        id32[0:65, 0:66])
                rv = rv_p.tile([128, 4], F32, tag="rv")
                nc.vector.reciprocal(rv[:], fi[:, 64:ST:128])
                ob = o_p.tile([128, 4, 64], F32, tag="ob")
                for qq in range(4):
                    nc.vector.tensor_scalar_mul(
                        ob[:, qq, :], fi[:, qq * 128:qq * 128 + 64],
                        rv[:, qq:qq + 1])
                nc.sync.dma_start(
                    out=out_d[st * ST:(st + 1) * ST,
                              h * 64:(h + 1) * 64].rearrange(
                                  "(q p) d -> p q d", p=128),
                    in_=ob[:])
    return nc


_NC_CACHE = None


def _host_consts():
    i = np.arange(0, D, 2, dtype=np.float64) / D          # 32 pair exponents
    freqs = 1.0 / (10000.0 ** i)                           # (32,)
    ang = np.arange(S, dtype=np.float64)[:, None] * freqs[None, :]  # (S, 32)
    cos = np.cos(ang).astype(np.float32)                   # (S, 32)
    sin = np.sin(ang).astype(np.float32)
    dcol = (np.arange(RC) % D) // 2                        # (320,) pair idx
    sinn = np.ascontiguousarray(sin[:, dcol])
    sinn[:, 0::2] *= -1.0                                  # pre-negate evens
    cosn = np.ascontiguousarray(cos[:, dcol]).astype(np.float16)  # (S, 320)
    sinn = sinn.astype(np.float16)
    kv, qq = np.meshgrid(np.arange(128), np.arange(128), indexing="ij")
    mask01 = (kv <= qq).astype(np.float16)                 # 1 = allowed
    ident16 = np.eye(128, dtype=np.float16)
    ident32 = np.eye(128, dtype=np.float32)
    return cosn, sinn, mask01, ident16, ident32


def _in_maps(x, Wq, Wk, Wv):
    x = np.asarray(x, dtype=np.float32).reshape(B * S, DIN)
    xt = np.ascontiguousarray(x.T).astype(np.float16)      # [DIN, B*S]
    Wq = np.asarray(Wq, dtype=np.float32)
    Wk = np.asarray(Wk, dtype=np.float32)
    Wv = np.asarray(Wv, dtype=np.float32)
    cosn, sinn, mask01, ident16, ident32 = _host_consts()

    in_maps = []
    for k in range(NCORES):
        w_all = np.hstack([
            Wq[:, k * 256:(k + 1) * 256],
            Wk[:, k * 64:(k + 1) * 64],
            Wv[:, k * 64:(k + 1) * 64],
        ]).astype(np.float16)
        in_maps.append({
            "xt": xt, "w": np.ascontiguousarray(w_all),
            "cosn": cosn, "sinn": sinn, "mask": mask01,
            "id16": ident16, "id32": ident32,
        })
    return in_maps


def _run(in_maps, **kwargs):
    global _NC_CACHE
    if _NC_CACHE is None:
        _NC_CACHE = build_bass()
        _NC_CACHE.finalize()
    return run_bass_kernel_spmd(_NC_CACHE, in_maps, list(range(NCORES)),
                                **kwargs)


def kernel(x, Wq, Wk, Wv):
    res = _run(_in_maps(x, Wq, Wk, Wv))
    out = np.concatenate([res.results[k]["out"] for k in range(NCORES)], axis=1)
    return out.reshape(B, S, 32 * D)


# revision 24
# speedup vs baseline: 1.4162x; 1.0081x over previous
"""GQA forward (b=2, s=2048, H=32 q heads, 8 kv heads, d=64) on 8 TRN2 cores.

Sharding: core k owns query heads 4k..4k+3 and kv head k. GQA group
structure makes attention fully local per core (q heads 4k..4k+3 attend
only to kv head k). x is replicated; W columns are sharded; outputs are
column-concatenated.

Per-core kernel (Tile framework), fp16 datapath / fp32 accumulation:
  - x.T is produced on the HOST (numpy transpose + fp16 cast) and DMA'd
    straight into SBUF — no on-chip transposes of x.
  - Projections in natural layout: QKV[s,384] = xT_chunk.T @ W_chunk
    accumulated in fp32 PSUM over 16 k-chunks (fp16 operands, 1 cyc/row).
  - RoPE on DVE with free-dim stride-2 views, fused with the PSUM->SBUF
    eviction (sin table pre-negated on host so plain tensor_tensor
    suffices); V columns go straight into the [V|1] resident.
  - Q/K flipped to [d, s] via PE transposes (fp16, 1 cyc/row).
  - Attention in transposed layout: S.T[kv,q] = K @ Q.T per 128-kv block,
    exp on ACT (scale=1/8 folded in) with fp16 output, causal handled by
    skipping blocks above the diagonal + multiplying the diagonal block
    of P by a 0/1 fp16 mask, ctx.T[65,q] = [V|1].T @ P.T accumulated in
    fp32 PSUM (row 64 = softmax sums).
  - Finalize: 4 PE transposes of ctx.T into one [128,4,66] PSUM bank,
    one reciprocal, 4 scalar muls, one DMA per (head, s-tile).
"""

import numpy as np
from contextlib import ExitStack

import concourse.bass as bass
import concourse.bacc as bacc
import concourse.mybir as mybir
from concourse import tile
from concourse.bass_utils import run_bass_kernel_spmd

F32 = mybir.dt.float32
F16 = mybir.dt.float16
MUL = mybir.AluOpType.mult
ADD = mybir.AluOpType.add

B = 2
S = 2048
DIN = 2048
D = 64              # head dim
HPC = 4             # query heads per core
NCORES = 8
WCOLS = 4 * D + D + D  # 256 q cols + 64 k + 64 v = 384
RC = 320            # roped columns (4 q heads + k head)
ST = 512            # s-tile (rows per outer step)
NST = B * S // ST   # 8 s-tiles
NCH = DIN // 128    # 16 k-chunks
NKV = S // 128      # kv tiles per batch


def build_bass():
    nc = bacc.Bacc(None, target_bir_lowering=False)
    xt_d = nc.declare_dram_parameter("xt", [DIN, B * S], F16, isOutput=False)
    w_d = nc.declare_dram_parameter("w", [DIN, WCOLS], F16, isOutput=False)
    cos_d = nc.declare_dram_parameter("cosn", [S, RC], F16, isOutput=False)
    sin_d = nc.declare_dram_parameter("sinn", [S, RC], F16, isOutput=False)
    mask_d = nc.declare_dram_parameter("mask", [128, 128], F16, isOutput=False)
    id16_d = nc.declare_dram_parameter("id16", [128, 128], F16, isOutput=False)
    id32_d = nc.declare_dram_parameter("id32", [128, 128], F32, isOutput=False)
    out_d = nc.declare_dram_parameter("out", [B * S, HPC * D], F32, isOutput=True)

    with ExitStack() as ctx:
        tc = ctx.enter_context(tile.TileContext(nc))
        const = ctx.enter_context(tc.tile_pool(name="const", bufs=1))
        resid = ctx.enter_context(tc.tile_pool(name="resid", bufs=1))
        xt_p = ctx.enter_context(tc.tile_pool(name="xt", bufs=2))
        qn_p = ctx.enter_context(tc.tile_pool(name="qn", bufs=3))
        qt_p = ctx.enter_context(tc.tile_pool(name="qt", bufs=4))
        p_p = ctx.enter_context(tc.tile_pool(name="p", bufs=4))
        cx_p = ctx.enter_context(tc.tile_pool(name="cx", bufs=2))
        o_p = ctx.enter_context(tc.tile_pool(name="o", bufs=3))
        rv_p = ctx.enter_context(tc.tile_pool(name="rv", bufs=3))
        tp_ps = ctx.enter_context(tc.tile_pool(name="tp_ps", bufs=2, space="PSUM"))
        pr_ps = ctx.enter_context(tc.tile_pool(name="pr_ps", bufs=2, space="PSUM"))
        sc_ps = ctx.enter_context(tc.tile_pool(name="sc_ps", bufs=2, space="PSUM"))
        cx_ps = ctx.enter_context(tc.tile_pool(name="cx_ps", bufs=2, space="PSUM"))

        # constants on the SCALAR engine's DMA queue so the x.T tiles (sync
        # queue) stream in parallel — the first projection only waits for
        # w chunk-group 0 + xt chunk-group 0 instead of ~7MB of constants.
        w_sb = const.tile([128, NCH, WCOLS], F16)
        for cg in range(4):
            nc.scalar.dma_start(
                out=w_sb[:, cg * 4:(cg + 1) * 4, :],
                in_=w_d[cg * 512:(cg + 1) * 512, :].rearrange(
                    "(c p) n -> p c n", p=128))
        mask_sb = const.tile([128, 128], F16)
        nc.scalar.dma_start(out=mask_sb[:], in_=mask_d[:])
        id16 = const.tile([128, 128], F16)
        nc.scalar.dma_start(out=id16[:], in_=id16_d[:])
        id32 = const.tile([128, 128], F32)
        nc.scalar.dma_start(out=id32[:], in_=id32_d[:])
        # rope tables resident, t-major ([128, t, col]); both batches share.
        # chunk-group cg holds t=4cg..4cg+3, exactly what s-tile cg%4 needs.
        ctab = const.tile([128, NCH, RC], F16)
        stab = const.tile([128, NCH, RC], F16)
        for cg in range(4):
            nc.scalar.dma_start(
                out=ctab[:, cg * 4:(cg + 1) * 4, :],
                in_=cos_d[cg * 512:(cg + 1) * 512, :].rearrange(
                    "(t p) n -> p t n", p=128))
            nc.scalar.dma_start(
                out=stab[:, cg * 4:(cg + 1) * 4, :],
                in_=sin_d[cg * 512:(cg + 1) * 512, :].rearrange(
                    "(t p) n -> p t n", p=128))

        # rows 0-63: K.T (RoPE'd); rows 64-127: duplicate copy so that the
        # scores matmul lhsT can match either base partition of the Q halves
        kt_res = resid.tile([128, B * S], F16)
        vp_res = resid.tile([128, B * NKV, 128], F16)  # [V|1|0pad] kv-tiles
        nc.vector.memset(vp_res[:], 0.0)
        nc.vector.memset(vp_res[:, :, 64:65], 1.0)

        for st in range(NST):
            b, sti = divmod(st, 4)

            # ---- x.T tile straight from HBM (host-transposed), split so the
            # first chunk-group's projections can start before the rest land
            xt = xt_p.tile([128, NCH, ST], F16, tag="xt")
            for cg in range(4):
                nc.sync.dma_start(
                    out=xt[:, cg * 4:(cg + 1) * 4, :],
                    in_=xt_d[cg * 512:(cg + 1) * 512,
                             st * ST:(st + 1) * ST].rearrange(
                                 "(c p) s -> p c s", p=128))

            # ---- projections (natural layout) + RoPE + transposes ----
            qta = qt_p.tile([128, ST], F16, tag="qta")   # heads 0,1 as [d,s]
            qtb = qt_p.tile([128, ST], F16, tag="qtb")   # heads 2,3 as [d,s]
            for pt in range(4):
                t = sti * 4 + pt  # within-batch 128-row block index
                pp = pr_ps.tile([128, WCOLS], F32, tag="pp")
                for c in range(NCH):
                    nc.tensor.matmul(
                        pp[:], xt[:, c, pt * 128:(pt + 1) * 128],
                        w_sb[:, c, :], start=(c == 0), stop=(c == NCH - 1))
                qn = qn_p.tile([128, RC], F16, tag="qn")
                ts = qn_p.tile([128, RC], F32, tag="ts")
                # even cols: qe*c - qo*s ; odd cols: qo*c + qe*s
                # (sin table pre-negated on host in even columns)
                nc.vector.tensor_tensor(
                    ts[:, 0:RC:2], pp[:, 1:RC:2], stab[:, t, 0:RC:2], MUL)
                nc.vector.tensor_tensor(
                    ts[:, 1:RC:2], pp[:, 0:RC:2], stab[:, t, 1:RC:2], MUL)
                nc.vector.tensor_tensor(qn[:], pp[:, 0:RC], ctab[:, t, :], MUL)
                nc.vector.tensor_tensor(qn[:], qn[:], ts[:], ADD)
                # V columns: straight into the [V|1] resident ([kv, d] natural)
                nc.vector.tensor_copy(
                    vp_res[:, b * NKV + t, 0:64], pp[:, RC:WCOLS])
                # flip Q to [d, s]
                for cb in range(2):
                    tp = tp_ps.tile([128, 128], F16, tag="tp")
                    nc.tensor.transpose(
                        tp[:], qn[:, cb * 128:(cb + 1) * 128], id16[:])
                    dst = qta if cb == 0 else qtb
                    nc.vector.tensor_copy(
                        dst[:, pt * 128:(pt + 1) * 128], tp[:])
                # flip K ([128, 64] -> [64, 128])
                tpk = tp_ps.tile([128, 128], F16, tag="tp")
                nc.tensor.transpose(tpk[0:64, :], qn[:, 256:320], id16[:])
                nc.vector.tensor_copy(
                    kt_res[0:64, st * ST + pt * 128:st * ST + (pt + 1) * 128],
                    tpk[0:64, :])
            nc.sync.dma_start(
                out=kt_res[64:128, st * ST:(st + 1) * ST],
                in_=kt_res[0:64, st * ST:(st + 1) * ST])

            # ---- attention for the 4 heads of this q-tile ----
            # full kv blocks first, diagonal blocks (which need the extra
            # mask op between exp and ctx) last; js[0] is always w0=0 so the
            # start=True ctx matmul initializes the whole bank.
            js = list(range(4 * sti)) + \
                 [4 * sti, 4 * sti + 1, 4 * sti + 2, 4 * sti + 3]
            for h in range(HPC):
                p0 = (h % 2) * 64
                qh = (qta if h < 2 else qtb)[p0:p0 + 64, :]
                cxt = cx_ps.tile([128, ST], F32, tag="cxt")
                # each ctx matmul is emitted one step behind its scores so
                # the next scores matmul isn't queued behind a ctx that is
                # still waiting on its exp (the PE executes in order)
                pend = None  # (psb, w0, start, vp slot)
                for idx, j in enumerate(js):
                    off = 128 * j - 512 * sti
                    w0 = max(0, off)
                    sc = sc_ps.tile([128, ST], F32, tag="sc")
                    nc.tensor.matmul(
                        sc[:, w0:ST],
                        kt_res[p0:p0 + 64, b * S + j * 128:b * S + (j + 1) * 128],
                        qh[:, w0:ST], start=True, stop=True)
                    psb = p_p.tile([128, ST], F16, tag="psb")
                    nc.scalar.activation(
                        psb[:, w0:ST], sc[:, w0:ST],
                        mybir.ActivationFunctionType.Exp, scale=0.125)
                    if j >= 4 * sti:
                        # zero the upper-triangle of the diagonal block
                        nc.vector.tensor_tensor(
                            psb[:, off:off + 128], psb[:, off:off + 128],
                            mask_sb[:], MUL)
                    if pend is not None:
                        nc.tensor.matmul(
                            cxt[:, pend[1]:ST], vp_res[:, pend[3], :],
                            pend[0][:, pend[1]:ST], start=pend[2], stop=False)
                    pend = (psb, w0, idx == 0, b * NKV + j)
                nc.tensor.matmul(
                    cxt[:, pend[1]:ST], vp_res[:, pend[3], :],
                    pend[0][:, pend[1]:ST], start=pend[2], stop=True)
                cxs = cx_p.tile([65, ST], F32, tag="cxs")
                nc.vector.tensor_copy(cxs[:], cxt[0:65, :])
                # fi shares the cx_ps buffers (same tag/shape as cxt): ctx of
                # head h+1 reuses the buffer fi of head h-1 released
                fi = cx_ps.tile([128, ST], F32, tag="cxt")
                for qq in range(4):
                    nc.tensor.transpose(
                        fi[:, qq * 128:qq * 128 + 66],
                        cxs[:, qq * 128:(qq + 1) * 128],
                        id32[0:65, 0:66])
                rv = rv_p.tile([128, 4], F32, tag="rv")
                nc.vector.reciprocal(rv[:], fi[:, 64:ST:128])
                ob = o_p.tile([128, 4, 64], F32, tag="ob")
                for qq in range(4):
                    nc.vector.tensor_scalar_mul(
                        ob[:, qq, :], fi[:, qq * 128:qq * 128 + 64],
                        rv[:, qq:qq + 1])
                nc.sync.dma_start(
                    out=out_d[st * ST:(st + 1) * ST,
                              h * 64:(h + 1) * 64].rearrange(
                                  "(q p) d -> p q d", p=128),
                    in_=ob[:])
    return nc


_NC_CACHE = None


def _host_consts():
    i = np.arange(0, D, 2, dtype=np.float64) / D          # 32 pair exponents
    freqs = 1.0 / (10000.0 ** i)                           # (32,)
    ang = np.arange(S, dtype=np.float64)[:, None] * freqs[None, :]  # (S, 32)
    cos = np.cos(ang).astype(np.float32)                   # (S, 32)
    sin = np.sin(ang).astype(np.float32)
    dcol = (np.arange(RC) % D) // 2                        # (320,) pair idx
    sinn = np.ascontiguousarray(sin[:, dcol])
    sinn[:, 0::2] *= -1.0                                  # pre-negate evens
    cosn = np.ascontiguousarray(cos[:, dcol]).astype(np.float16)  # (S, 320)
    sinn = sinn.astype(np.float16)
    kv, qq = np.meshgrid(np.arange(128), np.arange(128), indexing="ij")
    mask01 = (kv <= qq).astype(np.float16)                 # 1 = allowed
    ident16 = np.eye(128, dtype=np.float16)
    ident32 = np.eye(128, dtype=np.float32)
    return cosn, sinn, mask01, ident16, ident32


def _in_maps(x, Wq, Wk, Wv):
    x = np.asarray(x, dtype=np.float32).reshape(B * S, DIN)
    xt = np.ascontiguousarray(x.T).astype(np.float16)      # [DIN, B*S]
    Wq = np.asarray(Wq, dtype=np.float32)
    Wk = np.asarray(Wk, dtype=np.float32)
    Wv = np.asarray(Wv, dtype=np.float32)
    cosn, sinn, mask01, ident16, ident32 = _host_consts()

    in_maps = []
    for k in range(NCORES):
        w_all = np.hstack([
            Wq[:, k * 256:(k + 1) * 256],
            Wk[:, k * 64:(k + 1) * 64],
            Wv[:, k * 64:(k + 1) * 64],
        ]).astype(np.float16)
        in_maps.append({
            "xt": xt, "w": np.ascontiguousarray(w_all),
            "cosn": cosn, "sinn": sinn, "mask": mask01,
            "id16": ident16, "id32": ident32,
        })
    return in_maps


def _run(in_maps, **kwargs):
    global _NC_CACHE
    if _NC_CACHE is None:
        _NC_CACHE = build_bass()
        _NC_CACHE.finalize()
    return run_bass_kernel_spmd(_NC_CACHE, in_maps, list(range(NCORES)),
                                **kwargs)


def kernel(x, Wq, Wk, Wv):
    res = _run(_in_maps(x, Wq, Wk, Wv))
    out = np.concatenate([res.results[k]["out"] for k in range(NCORES)], axis=1)
    return out.reshape(B, S, 32 * D)
